# revision 2
# baseline (speedup 1.0000x reference)
"""Trainium2 Bass kernel for AttnBlock: GroupNorm -> single-head attention -> out proj + residual.

Shapes: x [B=8, C=512, L=2048].  Sharding: data-parallel over batch, one batch
element per NeuronCore (8 cores), no collectives.

Per-core dataflow ([C, L] = [512, 2048]), all matmuls bf16/fp8 with fp32 PSUM:
  1. GroupNorm(32 groups of 16ch): channel/group sums computed ON THE PE via
     one-hot group matmuls (G [128,32]) that track the x DMA chunk-by-chunk
     (also keeps HAM warm); x^2 chunks produced round-robin on DVE/ACT/GPSIMD
     and group-summed the same way.  Group stats broadcast back to channels
     via a tiny fp32 matmul with B = G^T.  h applied per tile on
     DVE/GPSIMD/ACT in parallel.
  2. q, k = WT.T @ h   ([co, l] layout, fp8 DoubleRow);  vT = h.T @ WvT
     ([l, co] layout).  PSUM drains alternate ACT/DVE so the PE never waits.
  3. Attention per 1024-wide query superblock:
       S^T[j, i] = sum_c k[c,j] q[c,i]  ->  PT = exp(scale*S^T)  (ACT, fp8)
       aT_un[i, c] = sum_j PT[j,i] vT[j,c];  rowsum via incremental running
       sums of PT pairs (DVE+GPSIMD chains) + one tiny ones-matmul per
       i-block;  aT = aT_un / rowsum;  a[c, i] via PE transpose (pipelined
       one i-block behind AV).
  4. o = WoT.T @ a + bo_eff + x (bf16 x reused from the GN load - no second
     fp32 x fetch);  output projection software-pipelined behind the next
     superblock's S^T, with the last quarter-blocks interleaved into the
     final AV loop to shrink the serial tail.
"""

import os
import sys

import numpy as np

if "/opt/trn_rl_repo" not in sys.path:
    sys.path.insert(0, "/opt/trn_rl_repo")

import ml_dtypes

B, C, L = 8, 512, 2048
NG = 32  # groups
GS = C // NG  # 16 channels per group
EPS = 1e-5
P = 128  # partitions
CT = C // P  # 4 channel tiles
LT = L // P  # 16 position tiles
SCALE = 1.0 / float(np.sqrt(C))

LAST_RESULT = None  # BassKernelResults of the most recent run (for test harness)


def _build_nc():
    import concourse.bass as bass
    from concourse import bacc, mybir, tile

    dt = mybir.dt
    f32, bf16, f8 = dt.float32, dt.bfloat16, dt.float8e4
    AF = mybir.ActivationFunctionType
    OP = mybir.AluOpType

    nc = bacc.Bacc()

    xbf_d = nc.declare_dram_parameter("xbf", [C, L], bf16, isOutput=False)
    wqT_d = nc.declare_dram_parameter("wqT", [P, 2, CT // 2, C], f8, isOutput=False)
    wkT_d = nc.declare_dram_parameter("wkT", [P, 2, CT // 2, C], f8, isOutput=False)
    wvT_d = nc.declare_dram_parameter("wvT", [P, 2, CT // 2, C], f8, isOutput=False)
    woT_d = nc.declare_dram_parameter("woT", [P, 2, CT // 2, C], f8, isOutput=False)
    cp_d = nc.declare_dram_parameter("cparams", [P, CT * 5], f32, isOutput=False)
    gmat_d = nc.declare_dram_parameter("gmat", [P, CT * NG], bf16, isOutput=False)
    bmat_d = nc.declare_dram_parameter("bmat", [NG, CT * P], f32, isOutput=False)
    out_d = nc.declare_dram_parameter("out", [C, L], f32, isOutput=True)

    ISUP_ = 1024
    NSUP_ = L // ISUP_  # 2
    NIB = ISUP_ // P  # 8 i-blocks per superblock
    XCH = 4  # x DMA chunks per tile (512 cols each)

    with tile.TileContext(nc) as tc:
        with (
            tc.tile_pool(name="consts", bufs=1) as consts,
            tc.tile_pool(name="xt", bufs=4) as xt_pool,
            tc.tile_pool(name="sq", bufs=4) as sq_pool,
            tc.tile_pool(name="ha", bufs=4) as ha_pool,
            tc.tile_pool(name="qk", bufs=2) as qk_pool,
            tc.tile_pool(name="vt", bufs=8) as vt_pool,
            tc.tile_pool(name="pt", bufs=17) as pt_pool,
            tc.tile_pool(name="w", bufs=1) as w_pool,
            tc.tile_pool(name="at", bufs=5) as at_pool,
            tc.tile_pool(name="ot", bufs=5) as ot_pool,
            tc.tile_pool(name="gn", bufs=4) as gn_pool,
            tc.tile_pool(name="ts", bufs=2) as ts_pool,
            tc.tile_pool(name="ps", bufs=2, space="PSUM") as ps_pool,
            tc.tile_pool(name="pa", bufs=2, space="PSUM") as pa_pool,
            tc.tile_pool(name="pr", bufs=2, space="PSUM") as pr_pool,
        ):
            # ---- constants ----
            epst = consts.tile([P, 1], f32, name="epst")
            nc.vector.memset(epst, float(EPS))
            sh_m2 = consts.tile([P, 1], f32, name="sh_m2")
            nc.vector.memset(sh_m2, -2.0)
            onesb = consts.tile([P, 1], bf16, name="onesb")
            nc.gpsimd.memset(onesb, 1.0)
            ident = consts.tile([P, P], bf16, name="ident")
            nc.gpsimd.memset(ident, 0.0)
            nc.gpsimd.affine_select(
                out=ident, in_=ident, compare_op=OP.not_equal, fill=1.0,
                base=0, pattern=[[-1, P]], channel_multiplier=1,
            )
            dummy = consts.tile([P, 512], bf16, name="dummy")
            nc.gpsimd.memset(dummy, 0.001)

            # ACT table preloads (Sqrt's table-load stalled the GN critical
            # path by ~1.3us when left to first use)
            tblscr = consts.tile([P, 1], f32, name="tblscr")
            nc.scalar.activation(out=tblscr, in_=epst, func=AF.Square)
            nc.scalar.activation(out=tblscr, in_=epst, func=AF.Sqrt, bias=epst, scale=1.0)
            nc.scalar.activation(out=tblscr, in_=epst, func=AF.Exp)

            # PE pre-warm: get the HAM activity window going before x lands
            def warm(n):
                wps = ps_pool.tile([P, 512], f32, name="warm", tag="s")
                for _ in range(n):
                    nc.tensor.matmul(wps, dummy[:, 0:128], dummy, start=True, stop=True)

            warm(6)

            # ---- DMA: x chunks first (stats critical path), then params ----
            x_t = []
            for t in range(CT):
                xt = xt_pool.tile([P, L], bf16, name=f"x{t}", tag="x")
                x_t.append(xt)
            for t in range(CT):
                for ch in range(XCH):
                    nc.sync.dma_start(
                        out=x_t[t][:, ch * 512 : (ch + 1) * 512],
                        in_=xbf_d[t * P : (t + 1) * P, ch * 512 : (ch + 1) * 512],
                    )
            gmat = consts.tile([P, CT * NG], bf16, name="gmat")
            nc.sync.dma_start(out=gmat, in_=gmat_d[:, :])
            bmat = consts.tile([NG, CT * P], f32, name="bmat")
            nc.sync.dma_start(out=bmat, in_=bmat_d[:, :])
            cpt = consts.tile([P, CT * 5], f32, name="cpt")
            nc.sync.dma_start(out=cpt, in_=cp_d[:, :])
            bq_t = [cpt[:, t * 5 + 0 : t * 5 + 1] for t in range(CT)]
            bk_t = [cpt[:, t * 5 + 1 : t * 5 + 2] for t in range(CT)]
            bo_t = [cpt[:, t * 5 + 2 : t * 5 + 3] for t in range(CT)]
            gam_t = [cpt[:, t * 5 + 3 : t * 5 + 4] for t in range(CT)]
            bet_t = [cpt[:, t * 5 + 4 : t * 5 + 5] for t in range(CT)]

            wq_all = w_pool.tile([P, 2, CT // 2, C], f8, name="wq_all", tag="wq")
            wk_all = w_pool.tile([P, 2, CT // 2, C], f8, name="wk_all", tag="wk")
            wv_all = w_pool.tile([P, 2, CT // 2, C], f8, name="wv_all", tag="wv")
            wo_all = w_pool.tile([P, 2, CT // 2, C], f8, name="wo_all", tag="wo2")
            for d_, wall in ((wqT_d, wq_all), (wkT_d, wk_all), (wvT_d, wv_all), (woT_d, wo_all)):
                nc.gpsimd.dma_start(out=wall, in_=d_[:, :, :, :])

            def w_slice(wall, cp, co):
                return wall[:, :, cp, co * P : (co + 1) * P]

            def w_rhs(wall, cp):
                return wall[:, :, cp, :]

            # ---- GroupNorm stats: PE group-sums tracking the x DMA ----
            sums_ps = pa_pool.tile([NG, 512], f32, name="gnsum", tag="pa")
            sqs_ps = pa_pool.tile([NG, 512], f32, name="gnsq", tag="pa")
            sq_eng = {0: nc.vector, 1: nc.scalar, 2: nc.gpsimd, 3: nc.scalar}
            sq_tiles = []
            chunks = [(t, ch) for t in range(CT) for ch in range(XCH)]

            def sum_mm(k):
                t, ch = chunks[k]
                nc.tensor.matmul(
                    sums_ps,
                    gmat[:, t * NG : (t + 1) * NG],
                    x_t[t][:, ch * 512 : (ch + 1) * 512],
                    start=(k == 0),
                    stop=(k == len(chunks) - 1),
                )

            def sq_make(k):
                t, ch = chunks[k]
                sq = sq_pool.tile([P, 512], bf16, name=f"sq{k}", tag="sq")
                eng = sq_eng[ch]
                xs = x_t[t][:, ch * 512 : (ch + 1) * 512]
                if eng is nc.scalar:
                    nc.scalar.activation(out=sq, in_=xs, func=AF.Square)
                else:
                    eng.tensor_mul(sq, xs, xs)
                sq_tiles.append(sq)

            def sq_mm(k):
                nc.tensor.matmul(
                    sqs_ps,
                    gmat[:, chunks[k][0] * NG : (chunks[k][0] + 1) * NG],
                    sq_tiles[k],
                    start=(k == 0),
                    stop=(k == len(chunks) - 1),
                )

            # lag the sq matmul one chunk behind the sum matmul so the PE
            # always has ready work
            for k in range(len(chunks)):
                sq_make(k)
                sum_mm(k)
                if k > 0:
                    sq_mm(k - 1)
            sq_mm(len(chunks) - 1)

            # ---- finalize stats -> per-channel scale/shift -> h ----
            red = gn_pool.tile([NG, 2], f32, name="red", tag="red")
            nc.vector.tensor_reduce(
                out=red[:, 0:1], in_=sums_ps, axis=mybir.AxisListType.X, op=OP.add
            )
            nc.vector.tensor_reduce(
                out=red[:, 1:2], in_=sqs_ps, axis=mybir.AxisListType.X, op=OP.add
            )
            gss = gn_pool.tile([NG, 2], f32, name="gss", tag="gss")
            nc.vector.tensor_scalar_mul(gss, red, float(1.0 / (GS * L)))
            nvar = gn_pool.tile([NG, 1], f32, name="nvar", tag="nv")
            nc.vector.scalar_tensor_tensor(
                out=nvar, in0=gss[:, 0:1], scalar=gss[:, 0:1],
                in1=gss[:, 1:2], op0=OP.mult, op1=OP.subtract,
            )
            stat2 = gn_pool.tile([NG, 2], f32, name="stat2", tag="st2")
            nc.scalar.activation(
                out=stat2[:, 0:1], in_=nvar, func=AF.Sqrt, bias=epst[0:NG, :], scale=-1.0
            )
            nc.vector.reciprocal(out=stat2[:, 0:1], in_=stat2[:, 0:1])
            # stat2[:,1] = -mu * rstd
            nc.vector.tensor_scalar(
                out=stat2[:, 1:2], in0=gss[:, 0:1], scalar1=stat2[:, 0:1],
                scalar2=-1.0, op0=OP.mult, op1=OP.mult,
            )

            h_t = []
            h_eng = {0: nc.vector, 1: nc.gpsimd, 2: nc.scalar, 3: nc.vector}
            for t in range(CT):
                bps = ps_pool.tile([P, 2], f32, name=f"bps{t}", tag="s")
                nc.tensor.matmul(
                    bps, bmat[:, t * P : (t + 1) * P], stat2, start=True, stop=True
                )
                sc = gn_pool.tile([P, 1], f32, name=f"sc{t}", tag="sc")
                nc.vector.tensor_mul(sc, bps[:, 0:1], gam_t[t])
                bc = gn_pool.tile([P, 1], f32, name=f"bc{t}", tag="bc")
                nc.vector.scalar_tensor_tensor(
                    out=bc, in0=bps[:, 1:2], scalar=gam_t[t], in1=bet_t[t],
                    op0=OP.mult, op1=OP.add,
                )
                if t % 2 == 0:
                    hp = ha_pool.tile([P, 2, L], f8, name=f"h{t // 2}", tag="ha")
                    h_t.append(hp)
                hsl = h_t[t // 2][:, t % 2, :]
                eng = h_eng[t]
                if eng is nc.scalar:
                    nc.scalar.activation(out=hsl, in_=x_t[t], func=AF.Identity, bias=bc, scale=sc)
                else:
                    eng.tensor_scalar(
                        out=hsl, in0=x_t[t], scalar1=sc, scalar2=bc,
                        op0=OP.mult, op1=OP.add,
                    )

            # ---- Q, K projections: [co, l], paired fp8 for DoubleRow S^T ----
            q_t, k_t = [], []
            for cp in range(CT // 2):
                qt = qk_pool.tile([P, 2, L], f8, name=f"q{cp}", tag="q")
                kt = qk_pool.tile([P, 2, L], f8, name=f"k{cp}", tag="k")
                q_t.append(qt)
                k_t.append(kt)
            qk_blk = 0
            for wts, dst, bias in ((wq_all, q_t, bq_t), (wk_all, k_t, bk_t)):
                for co in range(CT):
                    for lg in range(2):
                        ps = ps_pool.tile([P, 1024], f32, name=f"pq{co}_{lg}", tag="s")
                        for cp in range(CT // 2):
                            for ih in range(2):
                                nc.tensor.matmul(
                                    ps[:, ih * 512 : (ih + 1) * 512],
                                    w_slice(wts, cp, co),
                                    h_t[cp][:, :, lg * 1024 + ih * 512 : lg * 1024 + (ih + 1) * 512],
                                    start=(cp == 0),
                                    stop=(cp == CT // 2 - 1),
                                    perf_mode=mybir.MatmulPerfMode.DoubleRow,
                                )
                        od = dst[co // 2][:, co % 2, lg * 1024 : (lg + 1) * 1024]
                        if qk_blk % 2 == 0:
                            nc.scalar.activation(
                                out=od, in_=ps, func=AF.Identity, bias=bias[co], scale=1.0
                            )
                        else:
                            nc.vector.tensor_scalar_add(out=od, in0=ps, scalar1=bias[co])
                        qk_blk += 1

            # ---- V^T projection: [l, co], paired fp8 for DoubleRow AV ----
            v_t = []
            for jp in range(LT // 2):
                vt = vt_pool.tile([P, 2, 512], f8, name=f"v{jp}", tag="v")
                v_t.append(vt)
            for lt in range(LT):
                ps = pa_pool.tile([P, 512], f32, name=f"pv{lt}", tag="pa")
                for cp in range(CT // 2):
                    nc.tensor.matmul(
                        ps,
                        h_t[cp][:, :, lt * P : (lt + 1) * P],
                        w_rhs(wv_all, cp),
                        start=(cp == 0),
                        stop=(cp == CT // 2 - 1),
                        perf_mode=mybir.MatmulPerfMode.DoubleRow,
                    )
                if lt % 2 == 0:
                    nc.vector.tensor_copy(v_t[lt // 2][:, lt % 2, :], ps)
                else:
                    nc.scalar.activation(
                        out=v_t[lt // 2][:, lt % 2, :], in_=ps, func=AF.Identity, scale=1.0
                    )

            # ---- attention + interleaved output projection ----
            a_t = []
            for cp in range(CT // 2):
                at = ha_pool.tile([P, 2, L], f8, name=f"a{cp}", tag="ha")
                a_t.append(at)

            # incremental running rowsum state per superblock: two parallel
            # chains (DVE + GPSIMD), combined once at superblock end
            run_st = {}

            def st_setup(sup):
                pts = []
                for jp in range(LT // 2):
                    ptp = pt_pool.tile([P, 2, ISUP_], f8, name=f"pt{sup}_{jp}", tag="pt")
                    pts.append(ptp)
                run_st[sup] = {0: None, 1: None}
                return pts

            def st_j(sup, pts, j):
                i0 = sup * ISUP_
                ps = ps_pool.tile([P, ISUP_], f32, name=f"pst{sup}_{j}", tag="s")
                for cp in range(CT // 2):
                    for ih in range(2):
                        nc.tensor.matmul(
                            ps[:, ih * 512 : (ih + 1) * 512],
                            k_t[cp][:, :, j * P : (j + 1) * P],
                            q_t[cp][:, :, i0 + ih * 512 : i0 + (ih + 1) * 512],
                            start=(cp == 0),
                            stop=(cp == CT // 2 - 1),
                            perf_mode=mybir.MatmulPerfMode.DoubleRow,
                        )
                # exp(scale*s - 2): shift keeps fp8 range safe, cancels in
                # the normalization
                nc.scalar.activation(
                    out=pts[j // 2][:, j % 2, :], in_=ps, func=AF.Exp,
                    scale=SCALE, bias=sh_m2,
                )
                if j % 2 == 1:
                    jp = j // 2
                    lane = jp % 2
                    eng = nc.vector if lane == 0 else nc.gpsimd
                    nt = ts_pool.tile(
                        [P, ISUP_], bf16, name=f"nt{sup}_{jp}", tag=f"pair{lane}", bufs=2
                    )
                    eng.tensor_add(nt, pts[jp][:, 0, :], pts[jp][:, 1, :])
                    prev = run_st[sup][lane]
                    if prev is None:
                        run_st[sup][lane] = nt
                    else:
                        nxt = ts_pool.tile(
                            [P, ISUP_], bf16, name=f"run{sup}_{jp}", tag=f"run{lane}", bufs=2
                        )
                        eng.tensor_add(nxt, prev, nt)
                        run_st[sup][lane] = nxt

            def st_finish(sup):
                tsum = ts_pool.tile([P, ISUP_], bf16, name=f"tsum{sup}", tag="tsum", bufs=2)
                nc.vector.tensor_add(tsum, run_st[sup][0], run_st[sup][1])
                return tsum

            def st_phase(sup):
                pts = st_setup(sup)
                for j in range(LT):
                    st_j(sup, pts, j)
                return pts, st_finish(sup)

            def transpose_out(sup, ib, at):
                iblk = sup * ISUP_ + ib * P
                for cc in range(CT):
                    ptr = pr_pool.tile([P, P], bf16, name=f"ptr{sup}_{ib}_{cc}", tag="ptr")
                    nc.tensor.transpose(ptr, at[:, cc * P : (cc + 1) * P], ident)
                    if cc % 2 == 0:
                        nc.vector.tensor_copy(
                            a_t[cc // 2][:, cc % 2, iblk : iblk + P], ptr
                        )
                    else:
                        nc.scalar.activation(
                            out=a_t[cc // 2][:, cc % 2, iblk : iblk + P],
                            in_=ptr, func=AF.Identity, scale=1.0,
                        )

            def av_phase(sup, pts, tsum_get, next_pts=None, o_hooks=None):
                prev = None
                tsum = None
                for ib in range(NIB):
                    if o_hooks is not None and ib in o_hooks:
                        o_hooks[ib]()
                    if next_pts is not None:
                        st_j(sup + 1, next_pts, 2 * ib)
                        st_j(sup + 1, next_pts, 2 * ib + 1)
                    if next_pts is None and ib % 2 == 1:
                        pa = ps_pool.tile([P, 512], f32, name=f"pa{sup}_{ib}", tag="s")
                    else:
                        pa = pa_pool.tile([P, 512], f32, name=f"pa{sup}_{ib}", tag="pa")
                    for jp in range(LT // 2):
                        nc.tensor.matmul(
                            pa,
                            pts[jp][:, :, ib * P : (ib + 1) * P],
                            v_t[jp],
                            start=(jp == 0),
                            stop=(jp == LT // 2 - 1),
                            perf_mode=mybir.MatmulPerfMode.DoubleRow,
                        )
                    if tsum is None:
                        tsum = tsum_get()
                    pr = ps_pool.tile([P, 1], f32, name=f"pr{sup}_{ib}", tag="s")
                    nc.tensor.matmul(
                        pr, tsum[:, ib * P : (ib + 1) * P], onesb,
                        start=True, stop=True,
                    )
                    rec = gn_pool.tile([P, 1], f32, name=f"rec{sup}_{ib}", tag="rec")
                    nc.vector.reciprocal(out=rec, in_=pr)
                    at = at_pool.tile([P, 512], bf16, name=f"aT{sup}_{ib}", tag="aT")
                    nc.scalar.activation(
                        out=at, in_=pa, func=AF.Identity, scale=rec
                    )
                    if prev is not None:
                        transpose_out(sup, ib - 1, prev)
                    prev = at
                transpose_out(sup, NIB - 1, prev)

            def o_block(sup, l0, width, use_s_pool):
                for co in range(CT):
                    if use_s_pool:
                        ps = ps_pool.tile([P, width], f32, name=f"po{l0}_{co}", tag="s")
                    else:
                        ps = pa_pool.tile([P, width], f32, name=f"po{l0}_{co}", tag="pa")
                    for cp in range(CT // 2):
                        nc.tensor.matmul(
                            ps,
                            w_slice(wo_all, cp, co),
                            a_t[cp][:, :, l0 : l0 + width],
                            start=(cp == 0),
                            stop=(cp == CT // 2 - 1),
                            perf_mode=mybir.MatmulPerfMode.DoubleRow,
                        )
                    ot = ot_pool.tile([P, width], f32, name=f"o{l0}_{co}", tag="o")
                    nc.vector.scalar_tensor_tensor(
                        out=ot, in0=ps, scalar=bo_t[co],
                        in1=x_t[co][:, l0 : l0 + width],
                        op0=OP.add, op1=OP.add,
                    )
                    nc.sync.dma_start(
                        out=out_d[co * P : (co + 1) * P, l0 : l0 + width],
                        in_=ot,
                    )

            # software pipeline: O-proj of sup0 runs while sup1's S^T keeps
            # the PE busy; sup1's O runs as quarter-blocks interleaved into
            # the final AV loop so the serial tail is one 256-wide block
            pts0, tsum0 = st_phase(0)
            pts1 = st_setup(1)
            av_phase(0, pts0, lambda: tsum0, next_pts=pts1)
            tsum1_box = {}

            def tsum1_get():
                if "t" not in tsum1_box:
                    tsum1_box["t"] = st_finish(1)
                return tsum1_box["t"]

            o_block(0, 0, 512, False)
            o_block(0, 512, 512, False)
            o_hooks = {
                3: lambda: o_block(1, 1024, 256, True),
                5: lambda: o_block(1, 1280, 256, True),
                7: lambda: o_block(1, 1536, 256, True),
            }
            av_phase(1, pts1, tsum1_get, o_hooks=o_hooks)
            o_block(1, 1792, 256, True)

    nc.compile()
    return nc


def _pair_pack(WT):
    """[C_in, C_out] -> [P, 2, CT//2, C_out] fp8, pairing ci-chunks (2cp, 2cp+1)."""
    w4 = WT.reshape(CT // 2, 2, P, C).transpose(2, 1, 0, 3)
    return np.ascontiguousarray(w4).astype(ml_dtypes.float8_e4m3)


def _prep_maps(inputs):
    x = np.asarray(inputs["x"], dtype=np.float32)
    Wq = np.asarray(inputs["Wq"], dtype=np.float32)
    Wk = np.asarray(inputs["Wk"], dtype=np.float32)
    Wv = np.asarray(inputs["Wv"], dtype=np.float32)
    Wo = np.asarray(inputs["Wo"], dtype=np.float32)
    bq = np.asarray(inputs["bq"], dtype=np.float32)
    bk = np.asarray(inputs["bk"], dtype=np.float32)
    bv = np.asarray(inputs["bv"], dtype=np.float32)
    bo = np.asarray(inputs["bo"], dtype=np.float32)
    gam = np.asarray(inputs["gn_gamma"], dtype=np.float32)
    bet = np.asarray(inputs["gn_beta"], dtype=np.float32)

    bo_eff = bo + Wo @ bv  # v-bias commutes through attention weights (rows sum to 1)

    cp_ctile = np.stack([bq, bk, bo_eff.astype(np.float32), gam, bet], axis=1)  # [C, 5]
    cparams = cp_ctile.reshape(CT, P, 5).transpose(1, 0, 2).reshape(P, CT * 5)

    # one-hot group matrices: gmat[p, t*NG+g] = 1 iff group(t*P+p) == g
    # (PE group-sum lhsT); bmat[g, t*P+p] same predicate (stats broadcast lhsT)
    gmat = np.zeros((P, CT * NG), dtype=np.float32)
    bmat = np.zeros((NG, CT * P), dtype=np.float32)
    for t in range(CT):
        for p in range(P):
            g = (t * P + p) // GS
            gmat[p, t * NG + g] = 1.0
            bmat[g, t * P + p] = 1.0

    shared = {
        "wqT": _pair_pack(Wq.T),
        "wkT": _pair_pack(Wk.T),
        "wvT": _pair_pack(Wv.T),
        "woT": _pair_pack(Wo.T),
        "cparams": np.ascontiguousarray(cparams, dtype=np.float32),
        "gmat": gmat.astype(ml_dtypes.bfloat16),
        "bmat": np.ascontiguousarray(bmat, dtype=np.float32),
    }
    in_maps = []
    for i in range(B):
        m = dict(shared)
        m["xbf"] = np.ascontiguousarray(x[i]).astype(ml_dtypes.bfloat16)
        in_maps.append(m)
    return in_maps


def _install_trace_hook():
    """The image's antenv lacks axon_hooks; recreate the shim so bass_utils
    can reach the NTFF profiler in libaxon_pjrt.so (for exec_time_ns)."""
    import types

    if "antenv.axon_hooks" in sys.modules:
        return True
    try:
        from trn_agent_boot.trn_boot import _ntff_profile_via_ctypes

        hook = _ntff_profile_via_ctypes("/opt/axon/libaxon_pjrt.so")
        if hook is None:
            return False
        mod = types.ModuleType("antenv.axon_hooks")
        mod._hook = hook
        mod.get_axon_ntff_profile_hook = lambda: mod._hook
        mod.set_axon_ntff_profile_hook = lambda h: setattr(mod, "_hook", h)
        sys.modules["antenv.axon_hooks"] = mod
        return True
    except Exception as e:  # pragma: no cover
        print(f"trace hook install failed: {e}", file=sys.stderr)
        return False


def kernel(**inputs):
    global LAST_RESULT
    from concourse import bass_utils
    from concourse.bass_utils import run_bass_kernel_spmd

    trace = os.environ.get("KERNEL_TRACE", "0") == "1"
    if trace:
        trace = _install_trace_hook()
        # skip the remote-bucket artifact upload; keep everything local
        bass_utils.upload_artifacts = lambda tmpdir: f"local://{tmpdir}"
    in_maps = _prep_maps(inputs)
    nc = _build_nc()
    res = run_bass_kernel_spmd(nc, in_maps, core_ids=list(range(B)), trace=trace)
    LAST_RESULT = res
    out = np.stack([np.asarray(res.results[i]["out"]) for i in range(B)], axis=0)
    return out.astype(np.float32)


# revision 8
# speedup vs baseline: 1.1331x; 1.1331x over previous
"""Trainium2 Bass kernel for AttnBlock: GroupNorm -> single-head attention -> out proj + residual.

Shapes: x [B=8, C=512, L=2048].  Sharding: data-parallel over batch, one batch
element per NeuronCore (8 cores), no collectives.

Per-core dataflow ([C, L] = [512, 2048]), all matmuls bf16/fp8 with fp32 PSUM:
  1. GroupNorm(32 groups of 16ch): channel/group sums computed ON THE PE via
     one-hot group matmuls (G [128,32]) that track the x DMA tile-by-tile
     (also keeps HAM warm); x^2 chunks produced on DVE/ACT and group-summed
     the same way.  Group stats broadcast back to channels via a tiny fp32
     matmul with B = G^T.  h applied per tile on DVE/ACT/GPSIMD in parallel.
  2. q, k = WT.T @ h   ([co, l] layout, fp8 DoubleRow);  vT = h.T @ WvT
     ([l, co+1] layout with a ones-channel appended).  PSUM drains alternate
     ACT/DVE so the PE never waits.  V-projection interleaved into the first
     superblock's S^T phase.
  3. Attention per 1024-wide query superblock:
       S^T[j, i] = sum_c k[c,j] q[c,i]  ->  PT = exp(scale*S^T)  (ACT, fp8)
       aT_un[i, c] = sum_j PT[j,i] vT[j,c] via two half-matmuls (256 + 257
       cols); the ones-channel of vT makes column 512 the softmax rowsum, so
       no separate reduction is needed;  aT = aT_un * (1/rowsum) on ACT;
       a[c, i] via PE transpose (pipelined one i-block behind AV).
  4. o = WoT.T @ a + bo_eff + x (bf16 x reused from the GN load - no second
     fp32 x fetch);  output projection software-pipelined behind the next
     superblock's S^T, with the last quarter-blocks interleaved into the
     final AV loop to shrink the serial tail.
"""

import os
import sys

import numpy as np

if "/opt/trn_rl_repo" not in sys.path:
    sys.path.insert(0, "/opt/trn_rl_repo")

import ml_dtypes

B, C, L = 8, 512, 2048
NG = 32  # groups
GS = C // NG  # 16 channels per group
EPS = 1e-5
P = 128  # partitions
CT = C // P  # 4 channel tiles
LT = L // P  # 16 position tiles
SCALE = 1.0 / float(np.sqrt(C))

LAST_RESULT = None  # BassKernelResults of the most recent run (for test harness)


def _build_nc():
    import concourse.bass as bass
    from concourse import bacc, mybir, tile

    dt = mybir.dt
    f32, bf16, f8 = dt.float32, dt.bfloat16, dt.float8e4
    AF = mybir.ActivationFunctionType
    OP = mybir.AluOpType

    nc = bacc.Bacc()

    xbf_d = nc.declare_dram_parameter("xbf", [C, L], bf16, isOutput=False)
    wqT_d = nc.declare_dram_parameter("wqT", [P, 2, CT // 2, C], f8, isOutput=False)
    wkT_d = nc.declare_dram_parameter("wkT", [P, 2, CT // 2, C], f8, isOutput=False)
    wvT_d = nc.declare_dram_parameter("wvT", [P, 2, CT // 2, C], f8, isOutput=False)
    woT_d = nc.declare_dram_parameter("woT", [P, 2, CT // 2, C], f8, isOutput=False)
    cp_d = nc.declare_dram_parameter("cparams", [P, CT * 5], f32, isOutput=False)
    gmat_d = nc.declare_dram_parameter("gmat", [P, CT * NG], bf16, isOutput=False)
    bmat_d = nc.declare_dram_parameter("bmat", [NG, CT * P], f32, isOutput=False)
    out_d = nc.declare_dram_parameter("out", [C, L], f32, isOutput=True)

    ISUP_ = 1024
    NIB = ISUP_ // P  # 8 i-blocks per superblock
    XCH = 4  # GN processing chunks per x tile (512 cols each)
    VW = 544  # v tile padded width (512 ch + 2 ones cols, 16B-aligned pair stride)

    with tile.TileContext(nc) as tc:
        with (
            tc.tile_pool(name="consts", bufs=1) as consts,
            tc.tile_pool(name="xt", bufs=4) as xt_pool,
            tc.tile_pool(name="sq", bufs=4) as sq_pool,
            tc.tile_pool(name="ha", bufs=4) as ha_pool,
            tc.tile_pool(name="qk", bufs=2) as qk_pool,
            tc.tile_pool(name="vt", bufs=8) as vt_pool,
            tc.tile_pool(name="pt", bufs=17) as pt_pool,
            tc.tile_pool(name="w", bufs=1) as w_pool,
            tc.tile_pool(name="at", bufs=5) as at_pool,
            tc.tile_pool(name="ot", bufs=5) as ot_pool,
            tc.tile_pool(name="gn", bufs=4) as gn_pool,
            tc.tile_pool(name="ps", bufs=3, space="PSUM") as ps_pool,
            tc.tile_pool(name="paa", bufs=2, space="PSUM") as paa_pool,
            tc.tile_pool(name="pab", bufs=1, space="PSUM") as pab_pool,
            tc.tile_pool(name="pr", bufs=2, space="PSUM") as pr_pool,
        ):
            # ---- constants ----
            epst = consts.tile([P, 1], f32, name="epst")
            nc.vector.memset(epst, float(EPS))
            sh_m2 = consts.tile([P, 1], f32, name="sh_m2")
            nc.vector.memset(sh_m2, -2.0)
            ident = consts.tile([P, P], bf16, name="ident")
            nc.gpsimd.memset(ident, 0.0)
            nc.gpsimd.affine_select(
                out=ident, in_=ident, compare_op=OP.not_equal, fill=1.0,
                base=0, pattern=[[-1, P]], channel_multiplier=1,
            )
            dummy = consts.tile([P, 512], bf16, name="dummy")
            nc.gpsimd.memset(dummy, 0.001)
            ones4 = consts.tile([P, 4], bf16, name="ones4")
            nc.gpsimd.memset(ones4, 1.0)

            # ACT table preloads (Sqrt's table-load stalled the GN critical
            # path by ~1.3us when left to first use)
            tblscr = consts.tile([P, 1], f32, name="tblscr")
            nc.scalar.activation(out=tblscr, in_=epst, func=AF.Square)
            nc.scalar.activation(out=tblscr, in_=epst, func=AF.Sqrt, bias=epst, scale=1.0)
            nc.scalar.activation(out=tblscr, in_=epst, func=AF.Exp)

            # ---- DMA: small params first (different queue), then x tiles ----
            gmat = consts.tile([P, CT * NG], bf16, name="gmat")
            nc.scalar.dma_start(out=gmat, in_=gmat_d[:, :])
            bmat = consts.tile([NG, CT * P], f32, name="bmat")
            nc.scalar.dma_start(out=bmat, in_=bmat_d[:, :])
            cpt = consts.tile([P, CT * 5], f32, name="cpt")
            nc.scalar.dma_start(out=cpt, in_=cp_d[:, :])
            bq_t = [cpt[:, t * 5 + 0 : t * 5 + 1] for t in range(CT)]
            bk_t = [cpt[:, t * 5 + 1 : t * 5 + 2] for t in range(CT)]
            bo_t = [cpt[:, t * 5 + 2 : t * 5 + 3] for t in range(CT)]
            gam_t = [cpt[:, t * 5 + 3 : t * 5 + 4] for t in range(CT)]
            bet_t = [cpt[:, t * 5 + 4 : t * 5 + 5] for t in range(CT)]

            # PE pre-warm: get the HAM activity window going before x lands
            def warm(n):
                wps = ps_pool.tile([P, 512], f32, name="warm", tag="s")
                for _ in range(n):
                    nc.tensor.matmul(wps, dummy[:, 0:128], dummy, start=True, stop=True)

            warm(6)

            x_t = []
            for t in range(CT):
                xt = xt_pool.tile([P, L], bf16, name=f"x{t}", tag="x")
                nc.sync.dma_start(out=xt, in_=xbf_d[t * P : (t + 1) * P, :])
                x_t.append(xt)

            # q/k weights immediately (needed ~4us after x); v/o weights gated
            # behind x so they don't steal HBM bandwidth from the stats path
            wq_all = w_pool.tile([P, 2, CT // 2, C], f8, name="wq_all", tag="wq")
            wk_all = w_pool.tile([P, 2, CT // 2, C], f8, name="wk_all", tag="wk")
            wv_all = w_pool.tile([P, 2, CT // 2, C], f8, name="wv_all", tag="wv")
            wo_all = w_pool.tile([P, 2, CT // 2, C], f8, name="wo_all", tag="wo2")
            nc.gpsimd.dma_start(out=wq_all, in_=wqT_d[:, :, :, :])
            nc.gpsimd.dma_start(out=wk_all, in_=wkT_d[:, :, :, :])
            wgate = consts.tile([1, 1], bf16, name="wgate")
            nc.gpsimd.tensor_copy(wgate, x_t[CT - 1][0:1, 0:1])
            nc.gpsimd.dma_start(out=wv_all, in_=wvT_d[:, :, :, :])
            nc.gpsimd.dma_start(out=wo_all, in_=woT_d[:, :, :, :])

            def w_slice(wall, cp, co):
                return wall[:, :, cp, co * P : (co + 1) * P]

            def w_rhs(wall, cp):
                return wall[:, :, cp, :]

            # ---- GroupNorm stats: PE group-sums tracking the x DMA ----
            sums_ps = ps_pool.tile([NG, 512], f32, name="gnsum", tag="s")
            sqs_ps = ps_pool.tile([NG, 512], f32, name="gnsq", tag="s")
            sq_eng = {0: nc.vector, 1: nc.scalar, 2: nc.vector, 3: nc.scalar}
            sq_tiles = []
            chunks = [(t, ch) for t in range(CT) for ch in range(XCH)]

            def sum_mm(k):
                t, ch = chunks[k]
                nc.tensor.matmul(
                    sums_ps,
                    gmat[:, t * NG : (t + 1) * NG],
                    x_t[t][:, ch * 512 : (ch + 1) * 512],
                    start=(k == 0),
                    stop=(k == len(chunks) - 1),
                )

            def sq_make(k):
                t, ch = chunks[k]
                sq = sq_pool.tile([P, 512], bf16, name=f"sq{k}", tag="sq")
                eng = sq_eng[ch]
                xs = x_t[t][:, ch * 512 : (ch + 1) * 512]
                if eng is nc.scalar:
                    nc.scalar.activation(out=sq, in_=xs, func=AF.Square)
                else:
                    eng.tensor_mul(sq, xs, xs)
                sq_tiles.append(sq)

            def sq_mm(k):
                nc.tensor.matmul(
                    sqs_ps,
                    gmat[:, chunks[k][0] * NG : (chunks[k][0] + 1) * NG],
                    sq_tiles[k],
                    start=(k == 0),
                    stop=(k == len(chunks) - 1),
                )

            # lag the sq matmul one chunk behind the sum matmul so the PE
            # always has ready work
            for k in range(len(chunks)):
                sq_make(k)
                sum_mm(k)
                if k > 0:
                    sq_mm(k - 1)
            sq_mm(len(chunks) - 1)

            # ---- finalize stats -> per-channel scale/shift -> h ----
            red = gn_pool.tile([NG, 2], f32, name="red", tag="red")
            nc.vector.tensor_reduce(
                out=red[:, 0:1], in_=sums_ps, axis=mybir.AxisListType.X, op=OP.add
            )
            nc.vector.tensor_reduce(
                out=red[:, 1:2], in_=sqs_ps, axis=mybir.AxisListType.X, op=OP.add
            )
            gss = gn_pool.tile([NG, 2], f32, name="gss", tag="gss")
            nc.vector.tensor_scalar_mul(gss, red, float(1.0 / (GS * L)))
            nvar = gn_pool.tile([NG, 1], f32, name="nvar", tag="nv")
            nc.vector.scalar_tensor_tensor(
                out=nvar, in0=gss[:, 0:1], scalar=gss[:, 0:1],
                in1=gss[:, 1:2], op0=OP.mult, op1=OP.subtract,
            )
            stat2 = gn_pool.tile([NG, 2], f32, name="stat2", tag="st2")
            nc.scalar.activation(
                out=stat2[:, 0:1], in_=nvar, func=AF.Sqrt, bias=epst[0:NG, :], scale=-1.0
            )
            nc.vector.reciprocal(out=stat2[:, 0:1], in_=stat2[:, 0:1])
            # stat2[:,1] = -mu * rstd
            nc.vector.tensor_scalar(
                out=stat2[:, 1:2], in0=gss[:, 0:1], scalar1=stat2[:, 0:1],
                scalar2=-1.0, op0=OP.mult, op1=OP.mult,
            )

            h_t = []
            h_eng = {0: nc.vector, 1: nc.scalar, 2: nc.gpsimd, 3: nc.vector}
            for t in range(CT):
                bps = ps_pool.tile([P, 2], f32, name=f"bps{t}", tag="s")
                nc.tensor.matmul(
                    bps, bmat[:, t * P : (t + 1) * P], stat2, start=True, stop=True
                )
                sc = gn_pool.tile([P, 1], f32, name=f"sc{t}", tag="sc")
                nc.vector.tensor_mul(sc, bps[:, 0:1], gam_t[t])
                bc = gn_pool.tile([P, 1], f32, name=f"bc{t}", tag="bc")
                nc.vector.scalar_tensor_tensor(
                    out=bc, in0=bps[:, 1:2], scalar=gam_t[t], in1=bet_t[t],
                    op0=OP.mult, op1=OP.add,
                )
                if t % 2 == 0:
                    hp = ha_pool.tile([P, 2, L], f8, name=f"h{t // 2}", tag="ha")
                    h_t.append(hp)
                hsl = h_t[t // 2][:, t % 2, :]
                eng = h_eng[t]
                if eng is nc.scalar:
                    nc.scalar.activation(out=hsl, in_=x_t[t], func=AF.Identity, bias=bc, scale=sc)
                else:
                    eng.tensor_scalar(
                        out=hsl, in0=x_t[t], scalar1=sc, scalar2=bc,
                        op0=OP.mult, op1=OP.add,
                    )

            # ---- Q, K projections: [co, l], paired fp8 for DoubleRow S^T ----
            q_t, k_t = [], []
            for cp in range(CT // 2):
                qt = qk_pool.tile([P, 2, L], f8, name=f"q{cp}", tag="q")
                kt = qk_pool.tile([P, 2, L], f8, name=f"k{cp}", tag="k")
                q_t.append(qt)
                k_t.append(kt)
            qk_blk = 0
            for wts, dst, bias in ((wq_all, q_t, bq_t), (wk_all, k_t, bk_t)):
                for co in range(CT):
                    for lg in range(2):
                        psh = [
                            ps_pool.tile([P, 512], f32, name=f"pq{co}_{lg}_{ih}", tag="s")
                            for ih in range(2)
                        ]
                        for cp in range(CT // 2):
                            for ih in range(2):
                                nc.tensor.matmul(
                                    psh[ih],
                                    w_slice(wts, cp, co),
                                    h_t[cp][:, :, lg * 1024 + ih * 512 : lg * 1024 + (ih + 1) * 512],
                                    start=(cp == 0),
                                    stop=(cp == CT // 2 - 1),
                                    perf_mode=mybir.MatmulPerfMode.DoubleRow,
                                )
                        for ih in range(2):
                            od = dst[co // 2][
                                :, co % 2, lg * 1024 + ih * 512 : lg * 1024 + (ih + 1) * 512
                            ]
                            if (qk_blk + ih) % 2 == 0:
                                nc.scalar.activation(
                                    out=od, in_=psh[ih], func=AF.Identity, bias=bias[co], scale=1.0
                                )
                            else:
                                nc.vector.tensor_scalar_add(out=od, in0=psh[ih], scalar1=bias[co])
                        qk_blk += 1

            # ---- V^T projection: [l, co+ones], paired fp8 for DoubleRow AV ----
            v_t = []
            for jp in range(LT // 2):
                vt = vt_pool.tile([P, 2, VW], f8, name=f"v{jp}", tag="v")
                for pi in range(2):
                    nc.scalar.activation(
                        out=vt[:, pi, 512:514], in_=ones4[:, 0:2], func=AF.Identity, scale=1.0
                    )
                v_t.append(vt)

            def v_proj(lt):
                ps = paa_pool.tile([P, 512], f32, name=f"pv{lt}", tag="paa")
                for cp in range(CT // 2):
                    nc.tensor.matmul(
                        ps,
                        h_t[cp][:, :, lt * P : (lt + 1) * P],
                        w_rhs(wv_all, cp),
                        start=(cp == 0),
                        stop=(cp == CT // 2 - 1),
                        perf_mode=mybir.MatmulPerfMode.DoubleRow,
                    )
                nc.vector.tensor_copy(v_t[lt // 2][:, lt % 2, 0:512], ps)

            # ---- attention + interleaved output projection ----
            a_t = []
            for cp in range(CT // 2):
                at = ha_pool.tile([P, 2, L], f8, name=f"a{cp}", tag="ha")
                a_t.append(at)

            def st_setup(sup):
                pts = []
                for jp in range(LT // 2):
                    ptp = pt_pool.tile([P, 2, ISUP_], f8, name=f"pt{sup}_{jp}", tag="pt")
                    pts.append(ptp)
                return pts

            def st_j(sup, pts, j):
                i0 = sup * ISUP_
                psh = [
                    ps_pool.tile([P, 512], f32, name=f"pst{sup}_{j}_{ih}", tag="s")
                    for ih in range(2)
                ]
                for cp in range(CT // 2):
                    for ih in range(2):
                        nc.tensor.matmul(
                            psh[ih],
                            k_t[cp][:, :, j * P : (j + 1) * P],
                            q_t[cp][:, :, i0 + ih * 512 : i0 + (ih + 1) * 512],
                            start=(cp == 0),
                            stop=(cp == CT // 2 - 1),
                            perf_mode=mybir.MatmulPerfMode.DoubleRow,
                        )
                # exp(scale*s - 2): shift keeps fp8 range safe, cancels in
                # the normalization
                for ih in range(2):
                    nc.scalar.activation(
                        out=pts[j // 2][:, j % 2, ih * 512 : (ih + 1) * 512],
                        in_=psh[ih], func=AF.Exp, scale=SCALE, bias=sh_m2,
                    )

            def transpose_out(sup, ib, at):
                iblk = sup * ISUP_ + ib * P
                for cc in range(CT):
                    ptr = pr_pool.tile([P, P], bf16, name=f"ptr{sup}_{ib}_{cc}", tag="ptr")
                    nc.tensor.transpose(ptr, at[:, cc * P : (cc + 1) * P], ident)
                    if cc % 2 == 0:
                        nc.vector.tensor_copy(
                            a_t[cc // 2][:, cc % 2, iblk : iblk + P], ptr
                        )
                    else:
                        nc.scalar.activation(
                            out=a_t[cc // 2][:, cc % 2, iblk : iblk + P],
                            in_=ptr, func=AF.Identity, scale=1.0,
                        )

            def av_phase(sup, pts, next_pts=None, o_hooks=None):
                prev = None
                for ib in range(NIB):
                    if o_hooks is not None and ib in o_hooks:
                        o_hooks[ib]()
                    if next_pts is not None:
                        st_j(sup + 1, next_pts, 2 * ib)
                        st_j(sup + 1, next_pts, 2 * ib + 1)
                    pa_a = paa_pool.tile([P, 256], f32, name=f"paa{sup}_{ib}", tag="paa")
                    pa_b = pab_pool.tile([P, 258], f32, name=f"pab{sup}_{ib}", tag="pab")
                    for jp in range(LT // 2):
                        lhs = pts[jp][:, :, ib * P : (ib + 1) * P]
                        nc.tensor.matmul(
                            pa_a, lhs, v_t[jp][:, :, 0:256],
                            start=(jp == 0), stop=(jp == LT // 2 - 1),
                            perf_mode=mybir.MatmulPerfMode.DoubleRow,
                        )
                        nc.tensor.matmul(
                            pa_b, lhs, v_t[jp][:, :, 256:514],
                            start=(jp == 0), stop=(jp == LT // 2 - 1),
                            perf_mode=mybir.MatmulPerfMode.DoubleRow,
                        )
                    rec = gn_pool.tile([P, 1], f32, name=f"rec{sup}_{ib}", tag="rec")
                    nc.vector.reciprocal(out=rec, in_=pa_b[:, 256:257])
                    at = at_pool.tile([P, 512], bf16, name=f"aT{sup}_{ib}", tag="aT")
                    nc.scalar.activation(
                        out=at[:, 0:256], in_=pa_a, func=AF.Identity, scale=rec
                    )
                    nc.scalar.activation(
                        out=at[:, 256:512], in_=pa_b[:, 0:256], func=AF.Identity, scale=rec
                    )
                    if prev is not None:
                        transpose_out(sup, ib - 1, prev)
                    prev = at
                transpose_out(sup, NIB - 1, prev)

            def o_block(l0, width, tag):
                pool = ps_pool if tag == "s" else paa_pool
                for co in range(CT):
                    ps = pool.tile([P, width], f32, name=f"po{l0}_{co}", tag=tag)
                    for cp in range(CT // 2):
                        nc.tensor.matmul(
                            ps,
                            w_slice(wo_all, cp, co),
                            a_t[cp][:, :, l0 : l0 + width],
                            start=(cp == 0),
                            stop=(cp == CT // 2 - 1),
                            perf_mode=mybir.MatmulPerfMode.DoubleRow,
                        )
                    ot = ot_pool.tile([P, width], f32, name=f"o{l0}_{co}", tag="o")
                    nc.vector.scalar_tensor_tensor(
                        out=ot, in0=ps, scalar=bo_t[co],
                        in1=x_t[co][:, l0 : l0 + width],
                        op0=OP.add, op1=OP.add,
                    )
                    nc.sync.dma_start(
                        out=out_d[co * P : (co + 1) * P, l0 : l0 + width],
                        in_=ot,
                    )

            # S^T superblock 0 with the V projection interleaved (V is
            # independent of S; it fills the PE while ACT drains the exps)
            pts0 = st_setup(0)
            for j in range(LT):
                st_j(0, pts0, j)
                v_proj(j)

            # software pipeline: O-proj of sup0 runs while sup1's S^T keeps
            # the PE busy; sup1's O runs as quarter-blocks interleaved into
            # the final AV loop so the serial tail is one 256-wide block
            pts1 = st_setup(1)
            av_phase(0, pts0, next_pts=pts1)
            o_block(0, 512, "paa")
            o_block(512, 512, "paa")
            o_hooks = {
                3: lambda: o_block(1024, 256, "s"),
                5: lambda: o_block(1280, 256, "s"),
                7: lambda: o_block(1536, 256, "s"),
            }
            av_phase(1, pts1, o_hooks=o_hooks)
            o_block(1792, 256, "s")

    nc.compile()
    return nc


def _pair_pack(WT):
    """[C_in, C_out] -> [P, 2, CT//2, C_out] fp8, pairing ci-chunks (2cp, 2cp+1)."""
    w4 = WT.reshape(CT // 2, 2, P, C).transpose(2, 1, 0, 3)
    return np.ascontiguousarray(w4).astype(ml_dtypes.float8_e4m3)


def _prep_maps(inputs):
    x = np.asarray(inputs["x"], dtype=np.float32)
    Wq = np.asarray(inputs["Wq"], dtype=np.float32)
    Wk = np.asarray(inputs["Wk"], dtype=np.float32)
    Wv = np.asarray(inputs["Wv"], dtype=np.float32)
    Wo = np.asarray(inputs["Wo"], dtype=np.float32)
    bq = np.asarray(inputs["bq"], dtype=np.float32)
    bk = np.asarray(inputs["bk"], dtype=np.float32)
    bv = np.asarray(inputs["bv"], dtype=np.float32)
    bo = np.asarray(inputs["bo"], dtype=np.float32)
    gam = np.asarray(inputs["gn_gamma"], dtype=np.float32)
    bet = np.asarray(inputs["gn_beta"], dtype=np.float32)

    bo_eff = bo + Wo @ bv  # v-bias commutes through attention weights (rows sum to 1)

    cp_ctile = np.stack([bq, bk, bo_eff.astype(np.float32), gam, bet], axis=1)  # [C, 5]
    cparams = cp_ctile.reshape(CT, P, 5).transpose(1, 0, 2).reshape(P, CT * 5)

    # one-hot group matrices: gmat[p, t*NG+g] = 1 iff group(t*P+p) == g
    # (PE group-sum lhsT); bmat[g, t*P+p] same predicate (stats broadcast lhsT)
    gmat = np.zeros((P, CT * NG), dtype=np.float32)
    bmat = np.zeros((NG, CT * P), dtype=np.float32)
    for t in range(CT):
        for p in range(P):
            g = (t * P + p) // GS
            gmat[p, t * NG + g] = 1.0
            bmat[g, t * P + p] = 1.0

    shared = {
        "wqT": _pair_pack(Wq.T),
        "wkT": _pair_pack(Wk.T),
        "wvT": _pair_pack(Wv.T),
        "woT": _pair_pack(Wo.T),
        "cparams": np.ascontiguousarray(cparams, dtype=np.float32),
        "gmat": gmat.astype(ml_dtypes.bfloat16),
        "bmat": np.ascontiguousarray(bmat, dtype=np.float32),
    }
    in_maps = []
    for i in range(B):
        m = dict(shared)
        m["xbf"] = np.ascontiguousarray(x[i]).astype(ml_dtypes.bfloat16)
        in_maps.append(m)
    return in_maps


def _install_trace_hook():
    """The image's antenv lacks axon_hooks; recreate the shim so bass_utils
    can reach the NTFF profiler in libaxon_pjrt.so (for exec_time_ns)."""
    import types

    if "antenv.axon_hooks" in sys.modules:
        return True
    try:
        from trn_agent_boot.trn_boot import _ntff_profile_via_ctypes

        hook = _ntff_profile_via_ctypes("/opt/axon/libaxon_pjrt.so")
        if hook is None:
            return False
        mod = types.ModuleType("antenv.axon_hooks")
        mod._hook = hook
        mod.get_axon_ntff_profile_hook = lambda: mod._hook
        mod.set_axon_ntff_profile_hook = lambda h: setattr(mod, "_hook", h)
        sys.modules["antenv.axon_hooks"] = mod
        return True
    except Exception as e:  # pragma: no cover
        print(f"trace hook install failed: {e}", file=sys.stderr)
        return False


def kernel(**inputs):
    global LAST_RESULT
    from concourse import bass_utils
    from concourse.bass_utils import run_bass_kernel_spmd

    trace = os.environ.get("KERNEL_TRACE", "0") == "1"
    if trace:
        trace = _install_trace_hook()
        # skip the remote-bucket artifact upload; keep everything local
        bass_utils.upload_artifacts = lambda tmpdir: f"local://{tmpdir}"
    in_maps = _prep_maps(inputs)
    nc = _build_nc()
    res = run_bass_kernel_spmd(nc, in_maps, core_ids=list(range(B)), trace=trace)
    LAST_RESULT = res
    out = np.stack([np.asarray(res.results[i]["out"]) for i in range(B)], axis=0)
    return out.astype(np.float32)


# revision 14
# speedup vs baseline: 1.1777x; 1.0394x over previous
"""Trainium2 Bass kernel for AttnBlock: GroupNorm -> single-head attention -> out proj + residual.

Shapes: x [B=8, C=512, L=2048].  Sharding: data-parallel over batch, one batch
element per NeuronCore (8 cores), no collectives.

Per-core dataflow ([C, L] = [512, 2048]), all matmuls bf16/fp8 with fp32 PSUM:
  1. GroupNorm(32 groups of 16ch): channel/group sums computed ON THE PE via
     one-hot group matmuls (G [128,32]) that track the x DMA tile-by-tile
     (also keeps HAM warm); x^2 chunks produced on DVE/ACT and group-summed
     the same way.  Group stats broadcast back to channels via a tiny fp32
     matmul with B = G^T.  h applied per tile on DVE/ACT/GPSIMD in parallel.
  2. q, k = WT.T @ h   ([co, l] layout, fp8 DoubleRow);  vT = h.T @ WvT
     ([l, co+1] layout with a ones-channel appended).  PSUM drains alternate
     ACT/DVE so the PE never waits.  V-projection interleaved into the first
     superblock's S^T phase.
  3. Attention per 1024-wide query superblock:
       S^T[j, i] = sum_c k[c,j] q[c,i]  ->  PT = exp(scale*S^T)  (ACT, fp8)
       aT_un[i, c] = sum_j PT[j,i] vT[j,c] via two half-matmuls (256 + 257
       cols); the ones-channel of vT makes column 512 the softmax rowsum, so
       no separate reduction is needed;  aT = aT_un * (1/rowsum) on ACT;
       a[c, i] via PE transpose (pipelined one i-block behind AV).
  4. o = WoT.T @ a + bo_eff + x (bf16 x reused from the GN load - no second
     fp32 x fetch);  output projection software-pipelined behind the next
     superblock's S^T, with the last quarter-blocks interleaved into the
     final AV loop to shrink the serial tail.
"""

import os
import sys

import numpy as np

if "/opt/trn_rl_repo" not in sys.path:
    sys.path.insert(0, "/opt/trn_rl_repo")

import ml_dtypes

B, C, L = 8, 512, 2048
NG = 32  # groups
GS = C // NG  # 16 channels per group
EPS = 1e-5
P = 128  # partitions
CT = C // P  # 4 channel tiles
LT = L // P  # 16 position tiles
SCALE = 1.0 / float(np.sqrt(C))

LAST_RESULT = None  # BassKernelResults of the most recent run (for test harness)


def _build_nc():
    import concourse.bass as bass
    from concourse import bacc, mybir, tile

    dt = mybir.dt
    f32, bf16, f8 = dt.float32, dt.bfloat16, dt.float8e4
    AF = mybir.ActivationFunctionType
    OP = mybir.AluOpType

    nc = bacc.Bacc()

    xbf_d = nc.declare_dram_parameter("xbf", [C, L], bf16, isOutput=False)
    wqT_d = nc.declare_dram_parameter("wqT", [P, 2, CT // 2, C], f8, isOutput=False)
    wkT_d = nc.declare_dram_parameter("wkT", [P, 2, CT // 2, C], f8, isOutput=False)
    wvT_d = nc.declare_dram_parameter("wvT", [P, 2, CT // 2, C], f8, isOutput=False)
    woT_d = nc.declare_dram_parameter("woT", [P, 2, CT // 2, C], f8, isOutput=False)
    cp_d = nc.declare_dram_parameter("cparams", [P, CT * 5], f32, isOutput=False)
    gmat_d = nc.declare_dram_parameter("gmat", [P, CT * NG], bf16, isOutput=False)
    bmat_d = nc.declare_dram_parameter("bmat", [NG, CT * P], f32, isOutput=False)
    out_d = nc.declare_dram_parameter("out", [C, L], f32, isOutput=True)

    ISUP_ = 1024
    NIB = ISUP_ // P  # 8 i-blocks per superblock
    XCH = 4  # GN processing chunks per x tile (512 cols each)
    VW = 544  # v tile padded width (512 ch + 2 ones cols, 16B-aligned pair stride)

    with tile.TileContext(nc) as tc:
        with (
            tc.tile_pool(name="consts", bufs=1) as consts,
            tc.tile_pool(name="xt", bufs=4) as xt_pool,
            tc.tile_pool(name="sq", bufs=4) as sq_pool,
            tc.tile_pool(name="ha", bufs=4) as ha_pool,
            tc.tile_pool(name="qk", bufs=2) as qk_pool,
            tc.tile_pool(name="vt", bufs=8) as vt_pool,
            tc.tile_pool(name="pt", bufs=17) as pt_pool,
            tc.tile_pool(name="w", bufs=1) as w_pool,
            tc.tile_pool(name="at", bufs=5) as at_pool,
            tc.tile_pool(name="ot", bufs=5) as ot_pool,
            tc.tile_pool(name="gn", bufs=4) as gn_pool,
            tc.tile_pool(name="ps", bufs=3, space="PSUM") as ps_pool,
            tc.tile_pool(name="paa", bufs=2, space="PSUM") as paa_pool,
            tc.tile_pool(name="pab", bufs=1, space="PSUM") as pab_pool,
            tc.tile_pool(name="pr", bufs=2, space="PSUM") as pr_pool,
        ):
            # ---- constants ----
            epst = consts.tile([P, 1], f32, name="epst")
            nc.vector.memset(epst, float(EPS))
            sh_m2 = consts.tile([P, 1], f32, name="sh_m2")
            nc.vector.memset(sh_m2, -2.0)
            ident = consts.tile([P, P], bf16, name="ident")
            nc.gpsimd.memset(ident, 0.0)
            nc.gpsimd.affine_select(
                out=ident, in_=ident, compare_op=OP.not_equal, fill=1.0,
                base=0, pattern=[[-1, P]], channel_multiplier=1,
            )
            dummy = consts.tile([P, 512], bf16, name="dummy")
            nc.gpsimd.memset(dummy, 0.001)
            ones4 = consts.tile([P, 4], bf16, name="ones4")
            nc.gpsimd.memset(ones4, 1.0)

            # ACT table preloads: only Sqrt + Exp are ever used on ACT (each
            # first use costs a 1.3us table load; keep them off the critical
            # path and avoid any third function that would evict them)
            tblscr = consts.tile([P, 1], f32, name="tblscr")
            nc.scalar.activation(out=tblscr, in_=epst, func=AF.Sqrt, bias=epst, scale=1.0)
            nc.scalar.activation(out=tblscr, in_=epst, func=AF.Exp)

            # ---- DMA: small params first (different queue), then x tiles ----
            gmat = consts.tile([P, CT * NG], bf16, name="gmat")
            nc.scalar.dma_start(out=gmat, in_=gmat_d[:, :])
            bmat = consts.tile([NG, CT * P], f32, name="bmat")
            nc.scalar.dma_start(out=bmat, in_=bmat_d[:, :])
            cpt = consts.tile([P, CT * 5], f32, name="cpt")
            nc.scalar.dma_start(out=cpt, in_=cp_d[:, :])
            bq_t = [cpt[:, t * 5 + 0 : t * 5 + 1] for t in range(CT)]
            bk_t = [cpt[:, t * 5 + 1 : t * 5 + 2] for t in range(CT)]
            bo_t = [cpt[:, t * 5 + 2 : t * 5 + 3] for t in range(CT)]
            gam_t = [cpt[:, t * 5 + 3 : t * 5 + 4] for t in range(CT)]
            bet_t = [cpt[:, t * 5 + 4 : t * 5 + 5] for t in range(CT)]

            # PE pre-warm: get the HAM activity window going before x lands
            def warm(n):
                wps = ps_pool.tile([P, 512], f32, name="warm", tag="s")
                for _ in range(n):
                    nc.tensor.matmul(wps, dummy[:, 0:128], dummy, start=True, stop=True)

            warm(6)

            x_t = []
            for t in range(CT):
                xt = xt_pool.tile([P, L], bf16, name=f"x{t}", tag="x")
                x_t.append(xt)
            for t in range(CT):
                for hh in range(2):
                    nc.sync.dma_start(
                        out=x_t[t][:, hh * 1024 : (hh + 1) * 1024],
                        in_=xbf_d[t * P : (t + 1) * P, hh * 1024 : (hh + 1) * 1024],
                    )

            # q/k weights immediately (needed ~4us after x); v/o weights gated
            # behind x so they don't steal HBM bandwidth from the stats path
            wq_all = w_pool.tile([P, 2, CT // 2, C], f8, name="wq_all", tag="wq")
            wk_all = w_pool.tile([P, 2, CT // 2, C], f8, name="wk_all", tag="wk")
            wv_all = w_pool.tile([P, 2, CT // 2, C], f8, name="wv_all", tag="wv")
            wo_all = w_pool.tile([P, 2, CT // 2, C], f8, name="wo_all", tag="wo2")
            nc.gpsimd.dma_start(out=wq_all, in_=wqT_d[:, :, :, :])
            nc.gpsimd.dma_start(out=wk_all, in_=wkT_d[:, :, :, :])
            wgate = consts.tile([1, 1], bf16, name="wgate")
            nc.gpsimd.tensor_copy(wgate, x_t[CT - 1][0:1, 0:1])
            nc.gpsimd.dma_start(out=wv_all, in_=wvT_d[:, :, :, :])
            nc.gpsimd.dma_start(out=wo_all, in_=woT_d[:, :, :, :])

            def w_slice(wall, cp, co):
                return wall[:, :, cp, co * P : (co + 1) * P]

            def w_rhs(wall, cp):
                return wall[:, :, cp, :]

            # ---- GroupNorm stats: PE group-sums tracking the x DMA ----
            sums_ps = ps_pool.tile([NG, 512], f32, name="gnsum", tag="s")
            sqs_ps = ps_pool.tile([NG, 512], f32, name="gnsq", tag="s")
            # no ACT squares: ACT's table cache must stay {Sqrt, Exp}
            sq_eng = {0: nc.vector, 1: nc.gpsimd, 2: nc.vector, 3: nc.vector}
            sq_tiles = []
            chunks = [(t, ch) for t in range(CT) for ch in range(XCH)]

            def sum_mm(k):
                t, ch = chunks[k]
                nc.tensor.matmul(
                    sums_ps,
                    gmat[:, t * NG : (t + 1) * NG],
                    x_t[t][:, ch * 512 : (ch + 1) * 512],
                    start=(k == 0),
                    stop=(k == len(chunks) - 1),
                )

            def sq_make(k):
                t, ch = chunks[k]
                sq = sq_pool.tile([P, 512], bf16, name=f"sq{k}", tag="sq")
                xs = x_t[t][:, ch * 512 : (ch + 1) * 512]
                sq_eng[ch].tensor_mul(sq, xs, xs)
                sq_tiles.append(sq)

            def sq_mm(k):
                nc.tensor.matmul(
                    sqs_ps,
                    gmat[:, chunks[k][0] * NG : (chunks[k][0] + 1) * NG],
                    sq_tiles[k],
                    start=(k == 0),
                    stop=(k == len(chunks) - 1),
                )

            # lag the sq matmul one chunk behind the sum matmul so the PE
            # always has ready work
            for k in range(len(chunks)):
                sq_make(k)
                sum_mm(k)
                if k > 0:
                    sq_mm(k - 1)
            sq_mm(len(chunks) - 1)
            # PE filler bridging the stats-finalize window: keeps the HAM
            # activity monitor warm so QK doesn't start at half clock
            warm(8)

            # ---- finalize stats -> per-channel scale/shift -> h ----
            red = gn_pool.tile([NG, 2], f32, name="red", tag="red")
            nc.vector.tensor_reduce(
                out=red[:, 0:1], in_=sums_ps, axis=mybir.AxisListType.X, op=OP.add
            )
            nc.vector.tensor_reduce(
                out=red[:, 1:2], in_=sqs_ps, axis=mybir.AxisListType.X, op=OP.add
            )
            gss = gn_pool.tile([NG, 2], f32, name="gss", tag="gss")
            nc.vector.tensor_scalar_mul(gss, red, float(1.0 / (GS * L)))
            nvar = gn_pool.tile([NG, 1], f32, name="nvar", tag="nv")
            nc.vector.scalar_tensor_tensor(
                out=nvar, in0=gss[:, 0:1], scalar=gss[:, 0:1],
                in1=gss[:, 1:2], op0=OP.mult, op1=OP.subtract,
            )
            stat2 = gn_pool.tile([NG, 2], f32, name="stat2", tag="st2")
            nc.scalar.activation(
                out=stat2[:, 0:1], in_=nvar, func=AF.Sqrt, bias=epst[0:NG, :], scale=-1.0
            )
            nc.vector.reciprocal(out=stat2[:, 0:1], in_=stat2[:, 0:1])
            # stat2[:,1] = -mu * rstd
            nc.vector.tensor_scalar(
                out=stat2[:, 1:2], in0=gss[:, 0:1], scalar1=stat2[:, 0:1],
                scalar2=-1.0, op0=OP.mult, op1=OP.mult,
            )

            h_t = []
            h_eng = {0: nc.vector, 1: nc.scalar, 2: nc.gpsimd, 3: nc.vector}
            for t in range(CT):
                bps = ps_pool.tile([P, 2], f32, name=f"bps{t}", tag="s")
                nc.tensor.matmul(
                    bps, bmat[:, t * P : (t + 1) * P], stat2, start=True, stop=True
                )
                sc = gn_pool.tile([P, 1], f32, name=f"sc{t}", tag="sc")
                nc.vector.tensor_mul(sc, bps[:, 0:1], gam_t[t])
                bc = gn_pool.tile([P, 1], f32, name=f"bc{t}", tag="bc")
                nc.vector.scalar_tensor_tensor(
                    out=bc, in0=bps[:, 1:2], scalar=gam_t[t], in1=bet_t[t],
                    op0=OP.mult, op1=OP.add,
                )
                if t % 2 == 0:
                    hp = ha_pool.tile([P, 2, L], f8, name=f"h{t // 2}", tag="ha")
                    h_t.append(hp)
                hsl = h_t[t // 2][:, t % 2, :]
                eng = h_eng[t]
                if eng is nc.scalar:
                    nc.scalar.activation(out=hsl, in_=x_t[t], func=AF.Identity, bias=bc, scale=sc)
                else:
                    eng.tensor_scalar(
                        out=hsl, in0=x_t[t], scalar1=sc, scalar2=bc,
                        op0=OP.mult, op1=OP.add,
                    )

            # ---- Q, K projections: [co, l], paired fp8 for DoubleRow S^T ----
            q_t, k_t = [], []
            for cp in range(CT // 2):
                qt = qk_pool.tile([P, 2, L], f8, name=f"q{cp}", tag="q")
                kt = qk_pool.tile([P, 2, L], f8, name=f"k{cp}", tag="k")
                q_t.append(qt)
                k_t.append(kt)
            qk_blk = 0
            for wts, dst, bias in ((wq_all, q_t, bq_t), (wk_all, k_t, bk_t)):
                for co in range(CT):
                    for lg in range(2):
                        psh = [
                            ps_pool.tile([P, 512], f32, name=f"pq{co}_{lg}_{ih}", tag="s")
                            for ih in range(2)
                        ]
                        for cp in range(CT // 2):
                            for ih in range(2):
                                nc.tensor.matmul(
                                    psh[ih],
                                    w_slice(wts, cp, co),
                                    h_t[cp][:, :, lg * 1024 + ih * 512 : lg * 1024 + (ih + 1) * 512],
                                    start=(cp == 0),
                                    stop=(cp == CT // 2 - 1),
                                    perf_mode=mybir.MatmulPerfMode.DoubleRow,
                                )
                        for ih in range(2):
                            od = dst[co // 2][
                                :, co % 2, lg * 1024 + ih * 512 : lg * 1024 + (ih + 1) * 512
                            ]
                            if (qk_blk + ih) % 2 == 0:
                                nc.scalar.activation(
                                    out=od, in_=psh[ih], func=AF.Identity, bias=bias[co], scale=1.0
                                )
                            else:
                                nc.vector.tensor_scalar_add(out=od, in0=psh[ih], scalar1=bias[co])
                        qk_blk += 1

            # ---- V^T projection: [l, co+ones], paired fp8 for DoubleRow AV ----
            v_t = []
            for jp in range(LT // 2):
                vt = vt_pool.tile([P, 2, VW], f8, name=f"v{jp}", tag="v")
                for pi in range(2):
                    nc.scalar.activation(
                        out=vt[:, pi, 512:514], in_=ones4[:, 0:2], func=AF.Identity, scale=1.0
                    )
                v_t.append(vt)

            def v_proj(lt):
                ps = paa_pool.tile([P, 512], f32, name=f"pv{lt}", tag="paa")
                for cp in range(CT // 2):
                    nc.tensor.matmul(
                        ps,
                        h_t[cp][:, :, lt * P : (lt + 1) * P],
                        w_rhs(wv_all, cp),
                        start=(cp == 0),
                        stop=(cp == CT // 2 - 1),
                        perf_mode=mybir.MatmulPerfMode.DoubleRow,
                    )
                nc.vector.tensor_copy(v_t[lt // 2][:, lt % 2, 0:512], ps)

            # ---- attention + interleaved output projection ----
            a_t = []
            for cp in range(CT // 2):
                at = ha_pool.tile([P, 2, L], f8, name=f"a{cp}", tag="ha")
                a_t.append(at)

            def st_setup(sup):
                pts = []
                for jp in range(LT // 2):
                    ptp = pt_pool.tile([P, 2, ISUP_], f8, name=f"pt{sup}_{jp}", tag="pt")
                    pts.append(ptp)
                return pts

            def st_j(sup, pts, j):
                i0 = sup * ISUP_
                psh = [
                    ps_pool.tile([P, 512], f32, name=f"pst{sup}_{j}_{ih}", tag="s")
                    for ih in range(2)
                ]
                for cp in range(CT // 2):
                    for ih in range(2):
                        nc.tensor.matmul(
                            psh[ih],
                            k_t[cp][:, :, j * P : (j + 1) * P],
                            q_t[cp][:, :, i0 + ih * 512 : i0 + (ih + 1) * 512],
                            start=(cp == 0),
                            stop=(cp == CT // 2 - 1),
                            perf_mode=mybir.MatmulPerfMode.DoubleRow,
                        )
                # exp(scale*s - 2): shift keeps fp8 range safe, cancels in
                # the normalization
                for ih in range(2):
                    nc.scalar.activation(
                        out=pts[j // 2][:, j % 2, ih * 512 : (ih + 1) * 512],
                        in_=psh[ih], func=AF.Exp, scale=SCALE, bias=sh_m2,
                    )

            def transpose_out(sup, ib, at):
                iblk = sup * ISUP_ + ib * P
                for cc in range(CT):
                    ptr = pr_pool.tile([P, P], bf16, name=f"ptr{sup}_{ib}_{cc}", tag="ptr")
                    nc.tensor.transpose(ptr, at[:, cc * P : (cc + 1) * P], ident)
                    if cc % 2 == 0:
                        nc.vector.tensor_copy(
                            a_t[cc // 2][:, cc % 2, iblk : iblk + P], ptr
                        )
                    else:
                        nc.scalar.activation(
                            out=a_t[cc // 2][:, cc % 2, iblk : iblk + P],
                            in_=ptr, func=AF.Identity, scale=1.0,
                        )

            def av_phase(sup, pts, next_pts=None, o_hooks=None):
                prev = None
                for ib in range(NIB):
                    if o_hooks is not None and ib in o_hooks:
                        o_hooks[ib]()
                    if next_pts is not None:
                        st_j(sup + 1, next_pts, 2 * ib)
                        st_j(sup + 1, next_pts, 2 * ib + 1)
                    pa_a = paa_pool.tile([P, 256], f32, name=f"paa{sup}_{ib}", tag="paa")
                    pa_b = pab_pool.tile([P, 258], f32, name=f"pab{sup}_{ib}", tag="pab")
                    for jp in range(LT // 2):
                        lhs = pts[jp][:, :, ib * P : (ib + 1) * P]
                        nc.tensor.matmul(
                            pa_a, lhs, v_t[jp][:, :, 0:256],
                            start=(jp == 0), stop=(jp == LT // 2 - 1),
                            perf_mode=mybir.MatmulPerfMode.DoubleRow,
                        )
                        nc.tensor.matmul(
                            pa_b, lhs, v_t[jp][:, :, 256:514],
                            start=(jp == 0), stop=(jp == LT // 2 - 1),
                            perf_mode=mybir.MatmulPerfMode.DoubleRow,
                        )
                    rec = gn_pool.tile([P, 1], f32, name=f"rec{sup}_{ib}", tag="rec")
                    nc.vector.reciprocal(out=rec, in_=pa_b[:, 256:257])
                    at = at_pool.tile([P, 512], bf16, name=f"aT{sup}_{ib}", tag="aT")
                    if ib % 2 == 0:
                        nc.scalar.activation(
                            out=at[:, 0:256], in_=pa_a, func=AF.Identity, scale=rec
                        )
                        nc.vector.tensor_scalar_mul(at[:, 256:512], pa_b[:, 0:256], rec)
                    else:
                        nc.vector.tensor_scalar_mul(at[:, 0:256], pa_a, rec)
                        nc.scalar.activation(
                            out=at[:, 256:512], in_=pa_b[:, 0:256], func=AF.Identity, scale=rec
                        )
                    if prev is not None:
                        transpose_out(sup, ib - 1, prev)
                    prev = at
                transpose_out(sup, NIB - 1, prev)

            def o_block(l0, width, tag):
                pool = ps_pool if tag == "s" else paa_pool
                for co in range(CT):
                    ps = pool.tile([P, width], f32, name=f"po{l0}_{co}", tag=tag)
                    for cp in range(CT // 2):
                        nc.tensor.matmul(
                            ps,
                            w_slice(wo_all, cp, co),
                            a_t[cp][:, :, l0 : l0 + width],
                            start=(cp == 0),
                            stop=(cp == CT // 2 - 1),
                            perf_mode=mybir.MatmulPerfMode.DoubleRow,
                        )
                    ot = ot_pool.tile([P, width], f32, name=f"o{l0}_{co}", tag="o")
                    nc.vector.scalar_tensor_tensor(
                        out=ot, in0=ps, scalar=bo_t[co],
                        in1=x_t[co][:, l0 : l0 + width],
                        op0=OP.add, op1=OP.add,
                    )
                    nc.sync.dma_start(
                        out=out_d[co * P : (co + 1) * P, l0 : l0 + width],
                        in_=ot,
                    )

            # S^T superblock 0 with the V projection interleaved (V is
            # independent of S; it fills the PE while ACT drains the exps)
            pts0 = st_setup(0)
            for j in range(LT):
                st_j(0, pts0, j)
                v_proj(j)

            # software pipeline: O-proj of sup0 runs while sup1's S^T keeps
            # the PE busy; sup1's O runs as quarter-blocks interleaved into
            # the final AV loop so the serial tail is one 256-wide block
            pts1 = st_setup(1)
            av_phase(0, pts0, next_pts=pts1)
            o_block(0, 512, "paa")
            o_block(512, 512, "paa")
            o_hooks = {
                3: lambda: o_block(1024, 256, "s"),
                5: lambda: o_block(1280, 256, "s"),
                7: lambda: o_block(1536, 256, "s"),
            }
            av_phase(1, pts1, o_hooks=o_hooks)
            o_block(1792, 256, "s")

    nc.compile()
    return nc


def _pair_pack(WT):
    """[C_in, C_out] -> [P, 2, CT//2, C_out] fp8, pairing ci-chunks (2cp, 2cp+1)."""
    w4 = WT.reshape(CT // 2, 2, P, C).transpose(2, 1, 0, 3)
    return np.ascontiguousarray(w4).astype(ml_dtypes.float8_e4m3)


def _prep_maps(inputs):
    x = np.asarray(inputs["x"], dtype=np.float32)
    Wq = np.asarray(inputs["Wq"], dtype=np.float32)
    Wk = np.asarray(inputs["Wk"], dtype=np.float32)
    Wv = np.asarray(inputs["Wv"], dtype=np.float32)
    Wo = np.asarray(inputs["Wo"], dtype=np.float32)
    bq = np.asarray(inputs["bq"], dtype=np.float32)
    bk = np.asarray(inputs["bk"], dtype=np.float32)
    bv = np.asarray(inputs["bv"], dtype=np.float32)
    bo = np.asarray(inputs["bo"], dtype=np.float32)
    gam = np.asarray(inputs["gn_gamma"], dtype=np.float32)
    bet = np.asarray(inputs["gn_beta"], dtype=np.float32)

    bo_eff = bo + Wo @ bv  # v-bias commutes through attention weights (rows sum to 1)

    cp_ctile = np.stack([bq, bk, bo_eff.astype(np.float32), gam, bet], axis=1)  # [C, 5]
    cparams = cp_ctile.reshape(CT, P, 5).transpose(1, 0, 2).reshape(P, CT * 5)

    # one-hot group matrices: gmat[p, t*NG+g] = 1 iff group(t*P+p) == g
    # (PE group-sum lhsT); bmat[g, t*P+p] same predicate (stats broadcast lhsT)
    gmat = np.zeros((P, CT * NG), dtype=np.float32)
    bmat = np.zeros((NG, CT * P), dtype=np.float32)
    for t in range(CT):
        for p in range(P):
            g = (t * P + p) // GS
            gmat[p, t * NG + g] = 1.0
            bmat[g, t * P + p] = 1.0

    shared = {
        "wqT": _pair_pack(Wq.T),
        "wkT": _pair_pack(Wk.T),
        "wvT": _pair_pack(Wv.T),
        "woT": _pair_pack(Wo.T),
        "cparams": np.ascontiguousarray(cparams, dtype=np.float32),
        "gmat": gmat.astype(ml_dtypes.bfloat16),
        "bmat": np.ascontiguousarray(bmat, dtype=np.float32),
    }
    in_maps = []
    for i in range(B):
        m = dict(shared)
        m["xbf"] = np.ascontiguousarray(x[i]).astype(ml_dtypes.bfloat16)
        in_maps.append(m)
    return in_maps


def _install_trace_hook():
    """The image's antenv lacks axon_hooks; recreate the shim so bass_utils
    can reach the NTFF profiler in libaxon_pjrt.so (for exec_time_ns)."""
    import types

    if "antenv.axon_hooks" in sys.modules:
        return True
    try:
        from trn_agent_boot.trn_boot import _ntff_profile_via_ctypes

        hook = _ntff_profile_via_ctypes("/opt/axon/libaxon_pjrt.so")
        if hook is None:
            return False
        mod = types.ModuleType("antenv.axon_hooks")
        mod._hook = hook
        mod.get_axon_ntff_profile_hook = lambda: mod._hook
        mod.set_axon_ntff_profile_hook = lambda h: setattr(mod, "_hook", h)
        sys.modules["antenv.axon_hooks"] = mod
        return True
    except Exception as e:  # pragma: no cover
        print(f"trace hook install failed: {e}", file=sys.stderr)
        return False


def kernel(**inputs):
    global LAST_RESULT
    from concourse import bass_utils
    from concourse.bass_utils import run_bass_kernel_spmd

    trace = os.environ.get("KERNEL_TRACE", "0") == "1"
    if trace:
        trace = _install_trace_hook()
        # skip the remote-bucket artifact upload; keep everything local
        bass_utils.upload_artifacts = lambda tmpdir: f"local://{tmpdir}"
    in_maps = _prep_maps(inputs)
    nc = _build_nc()
    res = run_bass_kernel_spmd(nc, in_maps, core_ids=list(range(B)), trace=trace)
    LAST_RESULT = res
    out = np.stack([np.asarray(res.results[i]["out"]) for i in range(B)], axis=0)
    return out.astype(np.float32)
